# revision 6
# baseline (speedup 1.0000x reference)
"""AWQ 4-bit quantized linear (x @ dequant(qweight)) on 8 NeuronCores.

Column-parallel tensor sharding: each core owns OUT_F/8 = 1376 output
columns (172 packed int32 columns of qweight/qzeros, 1376 columns of
scales); x is replicated (pre-transposed to [in, tok] so the contraction
dim lands on SBUF partitions).

Per-core kernel:
  - prologue: unpack all 4-bit zero-points once (int32 shift/and on DVE,
    int32->fp16 cast on GPSIMD), park z16[32, 1376] in a DRAM scratch so
    per-group rows can be DMA-broadcast across partitions (stride-0
    DRAM reads);
  - dequantize the whole [4096, 1376] fp16 weight shard into SBUF once:
    per 128-row k-block (== one quant group): DVE unpacks nibbles
    int32->int32, GPSIMD casts to fp16, DVE computes (iw - z) * s with
    DMA-broadcast z/s rows (bit-exact with the fp32-compute/fp16-round
    reference);
  - matmul: stream x.T token-macro tiles, PSUM-accumulate
    y[tok 128, out<=512] over the 32 k-blocks, ScalarE copies PSUM fp32
    -> SBUF fp16, DMA out.
Output gathered host-side by concatenating the 8 column shards.
"""

import numpy as np

import concourse.bass as bass
import concourse.mybir as mybir
import concourse.tile as tile
from concourse import bacc
from concourse._compat import axon_active
from concourse.tile_rust import add_dep_helper

FP16 = mybir.dt.float16
FP32 = mybir.dt.float32
I32 = mybir.dt.int32

P = 128
N_CORES = 8
IN_F = 4096
OUT_F = 11008
W_BIT = 4
PACK = 8               # nibbles per int32
GROUP = 128            # quant group size == k-block size
NG = IN_F // GROUP     # 32 k-blocks
TOK = 2 * 2048         # tokens

OSH = OUT_F // N_CORES     # 1376 out columns per core
OPACK = OSH // PACK        # 172 packed columns per core

SHIFT = mybir.AluOpType.logical_shift_right
AND = mybir.AluOpType.bitwise_and


def _chunks(width, step=512):
    out = []
    o = 0
    while o < width:
        out.append((o, min(step, width - o)))
        o += step
    return out


def build_program(tok=TOK, in_f=IN_F, osh=OSH, tok_macro=512):
    """Emit the SPMD per-core program. Returns the compiled Bacc module."""
    ng = in_f // GROUP
    opack = osh // PACK
    assert tok % tok_macro == 0 and tok_macro % P == 0

    nc = bacc.Bacc("TRN2", target_bir_lowering=False, debug=not axon_active())
    xt = nc.declare_dram_parameter("xt", [in_f, tok], FP16, isOutput=False)
    qw = nc.declare_dram_parameter("qw", [in_f, opack], I32, isOutput=False)
    qz = nc.declare_dram_parameter("qz", [ng, opack], I32, isOutput=False)
    sc = nc.declare_dram_parameter("sc", [ng, osh], FP16, isOutput=False)
    y = nc.declare_dram_parameter("y", [tok, osh], FP16, isOutput=True)
    zdram = nc.dram_tensor("zscratch", [ng, osh], FP16)

    n_macro = tok // tok_macro
    tt_per_macro = tok_macro // P
    chunks = _chunks(osh)

    with tile.TileContext(nc) as tc:
        with (
            tc.tile_pool(name="prpool", bufs=1) as prpool,
            tc.tile_pool(name="wpool", bufs=1) as wpool,
            tc.tile_pool(name="xpool", bufs=2) as xpool,
            tc.tile_pool(name="qwpool", bufs=2) as qwpool,
            tc.tile_pool(name="bpool", bufs=2) as bpool,
            tc.tile_pool(name="ipool", bufs=2) as ipool,
            tc.tile_pool(name="stpool", bufs=3) as stpool,
            tc.tile_pool(name="pspool", bufs=2, space="PSUM") as pspool,
        ):
            # ---- prologue: unpack zero-points, park in DRAM scratch ----
            qzt = prpool.tile([ng, opack], I32)
            nc.scalar.dma_start(qzt[:], qz[:])
            z32 = prpool.tile([ng, opack, PACK], I32)
            for s in range(PACK):
                nc.vector.tensor_scalar(z32[:, :, s], qzt[:], 4 * s, 15,
                                        SHIFT, AND)
            z16 = prpool.tile([ng, osh], FP16)
            nc.gpsimd.tensor_copy(z16[:], z32.rearrange("g c s -> g (c s)"))
            zwrite = nc.scalar.dma_start(zdram[:], z16[:])

            # ---- Phase 1: dequantize the whole W shard into SBUF ----
            wts = []
            for g in range(ng):
                qwt = qwpool.tile([P, opack], I32, tag="qw")
                nc.scalar.dma_start(qwt[:], qw[g * P:(g + 1) * P, :])
                sb = bpool.tile([P, osh], FP16, tag="sb")
                nc.scalar.dma_start(
                    sb[:], sc[g:g + 1, :].to_broadcast((P, osh)))
                zb = bpool.tile([P, osh], FP16, tag="zb")
                zread = nc.scalar.dma_start(
                    zb[:], zdram[g:g + 1, :].to_broadcast((P, osh)))
                add_dep_helper(zread.ins, zwrite.ins, sync=True,
                               reason="zscratch RAW")

                iw32 = ipool.tile([P, opack, PACK], I32, tag="iw32")
                for s in range(PACK):
                    nc.vector.tensor_scalar(iw32[:, :, s], qwt[:], 4 * s, 15,
                                            SHIFT, AND)
                iw16 = ipool.tile([P, osh], FP16, tag="iw16")
                nc.gpsimd.tensor_copy(
                    iw16[:], iw32.rearrange("p c s -> p (c s)"))

                tmp = ipool.tile([P, osh], FP16, tag="tmp")
                nc.vector.tensor_sub(tmp[:], iw16[:], zb[:])
                wt = wpool.tile([P, osh], FP16, tag=f"w{g}", name=f"w{g}")
                nc.vector.tensor_mul(wt[:], tmp[:], sb[:])
                wts.append(wt)

            # ---- Phase 2: matmul y[tok, osh] = x @ W ----
            for m in range(n_macro):
                t0 = m * tok_macro
                xts = []
                for k in range(ng):
                    xtile = xpool.tile([P, tok_macro], FP16, tag=f"x{k}",
                                       name=f"x{k}")
                    nc.sync.dma_start(
                        xtile[:], xt[k * P:(k + 1) * P, t0:t0 + tok_macro])
                    xts.append(xtile)
                for tt in range(tt_per_macro):
                    pss = [pspool.tile([P, 512], FP32, tag=f"ps{j}",
                                       name=f"ps{j}")
                           for j in range(len(chunks))]
                    for k in range(ng):
                        lhs = xts[k][:, tt * P:(tt + 1) * P]
                        for j, (o0, on) in enumerate(chunks):
                            nc.tensor.matmul(
                                pss[j][:, :on], lhs, wts[k][:, o0:o0 + on],
                                start=(k == 0), stop=(k == ng - 1))
                    st = stpool.tile([P, osh], FP16, tag="st")
                    for j, (o0, on) in enumerate(chunks):
                        nc.scalar.copy(st[:, o0:o0 + on], pss[j][:, :on])
                    r0 = t0 + tt * P
                    nc.scalar.dma_start(y[r0:r0 + P, :], st[:])

    nc.compile()
    return nc


_PROGRAM = None

# test-harness hooks (unused by the grading path)
TRACE = False
TRACE_KWARGS = {}
LAST_RESULT = None


def _get_program():
    global _PROGRAM
    if _PROGRAM is None:
        _PROGRAM = build_program()
    return _PROGRAM


def kernel(x, qweight, qzeros, scales):
    from concourse.bass_utils import run_bass_kernel_spmd

    x = np.asarray(x)
    qweight = np.asarray(qweight)
    qzeros = np.asarray(qzeros)
    scales = np.asarray(scales)

    xt = np.ascontiguousarray(x.reshape(TOK, IN_F).T)
    in_maps = []
    for c in range(N_CORES):
        in_maps.append({
            "xt": xt,
            "qw": np.ascontiguousarray(qweight[:, c * OPACK:(c + 1) * OPACK]),
            "qz": np.ascontiguousarray(qzeros[:, c * OPACK:(c + 1) * OPACK]),
            "sc": np.ascontiguousarray(scales[:, c * OSH:(c + 1) * OSH]),
        })

    nc = _get_program()
    res = run_bass_kernel_spmd(nc, in_maps, list(range(N_CORES)),
                               trace=TRACE, **TRACE_KWARGS)
    global LAST_RESULT
    LAST_RESULT = res
    y = np.concatenate([res.results[i]["y"] for i in range(N_CORES)], axis=1)
    return y.reshape(x.shape[0], x.shape[1], OUT_F)
